# revision 15
# baseline (speedup 1.0000x reference)
# Trainium2 Bass kernel for nn_DecoderMHA (dense decoder multi-head attention).
#
# Sharding (8 NeuronCores): batch (4) x tensor-parallel over heads (2).
# Core c handles batch b = c//2 and heads [tp*8, tp*8+8) where tp = c%2,
# i.e. a 512-wide slice of the QKV projection output dim and the matching
# 512 rows of Wo^T.  Per-core partial outputs (bf16) are summed on the host
# (y[b] = part[b,0] + part[b,1] + bo).
#
# Per-core pipeline (matmul operands bf16, fp32 PSUM accumulation):
#   A) Q^T/K^T [512,2048] and V [2048,512] projections from x^T.
#   B) Attention in transposed-scores layout, processed per head PAIR
#      (the two heads sharing a [128,seq] SBUF tile at partition rows
#      0:64 / 64:128).  Score matmuls for the pair are packed into
#      concurrent PE row-groups (tile_position auto-derived from
#      base_partition 0/64) and land in one [128,2,512] PSUM strip;
#      ScalarE applies exp(s/8 + pad_bias) for both heads in a single
#      activation op.  attn@V uses a ones-augmented V (65th column) so
#      PSUM row 64 of the [65,2,512] out tile is the softmax denominator.
#      Normalisation is fully on-chip: DVE copies the denominator rows to
#      SBUF, one stride-0 broadcast DMA replicates them across partitions,
#      and a DVE divide writes the normalised out^T in bf16.
#      Rounds are software-pipelined (scores of round r+1 issue before
#      attn@V of round r) so ScalarE - the attention-phase bottleneck -
#      never starves, and projection / output-projection work is pumped
#      into the PE gaps as fillers.
#   C) y = out @ Wo^T partial via out^T-as-lhsT matmuls, bf16 output.
import os
import numpy as np

BSZ, SEQ, DM = 4, 2048, 1024
HEADS, DK = 16, 64
NCORES, TP = 8, 2
E = DM // TP          # 512 per-core projection slice
HPC = HEADS // TP     # 8 heads per core
P = 128
NDC = DM // P         # 8 contraction chunks
NEC = E // P          # 4 e-chunks (head pairs)
NSC = SEQ // P        # 16 sequence chunks
NSB = SEQ // 512      # 4 sequence blocks
NQB = SEQ // 512      # 4 query blocks of 512
SCALE = 1.0 / float(np.sqrt(DK))

_CACHED = {}


def _split_sync_waits(nc, mybir, max_waits=1):
    """The walrus in this container only accepts one sync-wait per
    instruction; move excess waits onto NoOps in front."""
    n = 0
    for fn in nc.m.functions:
        for bb in fn.blocks:
            insts = bb.instructions
            i = 0
            while i < len(insts):
                inst = insts[i]
                si = getattr(inst, "sync_info", None)
                if si is not None and si.on_wait and len(si.on_wait) > max_waits:
                    waits = list(si.on_wait)
                    extra, keep = waits[:-max_waits], waits[-max_waits:]
                    si.on_wait = keep
                    pos = i
                    for j in range(0, len(extra), max_waits):
                        nop = mybir.InstNoOp(
                            name=nc.get_next_instruction_name(),
                            sync_info=mybir.SyncInfo(
                                on_wait=extra[j:j + max_waits], on_update=[]),
                            bass_nofuse=True,
                            engine=inst.engine,
                        )
                        insts.insert(pos, nop)
                        pos += 1
                        i += 1
                        n += 1
                i += 1
    return n


def _build():
    import concourse.bass as bass
    from concourse import mybir
    from concourse.tile import TileContext

    f32 = mybir.dt.float32
    bf16 = mybir.dt.bfloat16
    Exp = mybir.ActivationFunctionType.Exp
    MUL = mybir.AluOpType.mult
    ADD = mybir.AluOpType.add
    DIV = mybir.AluOpType.divide

    nc = bass.Bass("TRN2", target_bir_lowering=False, debug=False,
                   num_devices=NCORES)

    # DRAM I/O (per-core layouts, pre-tiled on host)
    xt = nc.dram_tensor("xt", [NSB, P, NDC, 512], bf16, kind="ExternalInput")
    wq = nc.dram_tensor("wq", [P, NDC, E], bf16, kind="ExternalInput")
    wk = nc.dram_tensor("wk", [P, NDC, E], bf16, kind="ExternalInput")
    wv = nc.dram_tensor("wv", [P, NDC, E], bf16, kind="ExternalInput")
    wo = nc.dram_tensor("wo", [P, NEC, DM], bf16, kind="ExternalInput")
    bqt = nc.dram_tensor("bqt", [P, NEC], f32, kind="ExternalInput")
    bkt = nc.dram_tensor("bkt", [P, NEC], f32, kind="ExternalInput")
    bvb = nc.dram_tensor("bvb", [P, E], f32, kind="ExternalInput")
    cm = nc.dram_tensor("cm", [P, P], f32, kind="ExternalInput")
    pb = nc.dram_tensor("pb", [P, NSC], f32, kind="ExternalInput")
    one1 = nc.dram_tensor("one1", [1, P], f32, kind="ExternalInput")
    y = nc.dram_tensor("y", [SEQ, DM], bf16, kind="ExternalOutput")

    with TileContext(nc) as tc:
        with (
            tc.tile_pool(name="persist", bufs=1) as pp,
            tc.tile_pool(name="psS", bufs=2, space="PSUM") as psS,
            tc.tile_pool(name="psO", bufs=2, space="PSUM") as psO,
        ):
            # ---- persistent SBUF ----
            qT = [pp.tile([P, SEQ], bf16, tag=f"qT{t}", name=f"qT{t}")
                  for t in range(NEC)]
            kT = [pp.tile([P, SEQ], bf16, tag=f"kT{t}", name=f"kT{t}")
                  for t in range(NEC)]
            vA = [pp.tile([P, HPC, DK + 1], bf16, tag=f"vA{g}",
                          name=f"vA{g}") for g in range(NSC)]
            outT = [pp.tile([P, SEQ], bf16, tag=f"oT{t}", name=f"oT{t}")
                    for t in range(NEC)]
            cm_s = pp.tile([P, P], f32, tag="cm")
            one1_s = pp.tile([1, P], f32, tag="one1")
            pb_s = pp.tile([P, NSC], f32, tag="pb")
            bq_s = pp.tile([P, NEC], f32, tag="bq")
            bk_s = pp.tile([P, NEC], f32, tag="bk")
            bv_s = pp.tile([P, E], f32, tag="bv")
            wq_s = pp.tile([P, NDC, E], bf16, tag="wq")
            wk_s = pp.tile([P, NDC, E], bf16, tag="wk")
            wv_s = pp.tile([P, NDC, E], bf16, tag="wv")
            wo_s = pp.tile([P, NEC, DM], bf16, tag="wo")
            xt_s = [pp.tile([P, NDC, 512], bf16, tag=f"xt{sb}",
                            name=f"xt{sb}") for sb in range(NSB)]

            # ---- input DMAs, in consumption order ----
            nc.sync.dma_start(wq_s[:], wq[:])
            nc.sync.dma_start(xt_s[0][:], xt[0])
            nc.sync.dma_start(wk_s[:], wk[:])
            nc.sync.dma_start(bq_s[:], bqt[:])
            nc.sync.dma_start(bk_s[:], bkt[:])
            nc.sync.dma_start(pb_s[:], pb[:])
            nc.sync.dma_start(cm_s[:], cm[:])
            nc.sync.dma_start(one1_s[:], one1[:])
            for sb in range(1, NSB):
                nc.sync.dma_start(xt_s[sb][:], xt[sb])
            nc.sync.dma_start(wv_s[:], wv[:])
            nc.sync.dma_start(bv_s[:], bvb[:])
            for g in range(NSC):
                nc.gpsimd.memset(vA[g][:, :, DK:DK + 1], 1.0)
            nc.sync.dma_start(wo_s[:], wo[:])

            # ---- emit helpers ----
            def qk_strip(t, dst, w_s, b_s, sb, nm):
                psum = psS.tile([P, 2, 512], f32, tag="strip",
                                name=f"p{nm}{t}_{sb}")
                for dc in range(NDC):
                    nc.tensor.matmul(
                        psum[:, 0, :],
                        w_s[:, dc, t * P:(t + 1) * P],
                        xt_s[sb][:, dc, :],
                        start=(dc == 0), stop=(dc == NDC - 1))
                nc.vector.tensor_tensor(
                    dst[t][:, sb * 512:(sb + 1) * 512],
                    psum[:, 0, :],
                    b_s[:, t:t + 1].to_broadcast([P, 512]),
                    ADD)

            def v_strip(g):
                sb, ssc = g // 4, g % 4
                psum = psS.tile([P, 2, 512], f32, tag="strip", name=f"pv{g}")
                for dc in range(NDC):
                    nc.tensor.matmul(
                        psum[:, 0, :],
                        xt_s[sb][:, dc, ssc * P:(ssc + 1) * P],
                        wv_s[:, dc, :],
                        start=(dc == 0), stop=(dc == NDC - 1))
                nc.vector.tensor_tensor(
                    vA[g][:, :, 0:DK],
                    psum[:, 0, :].rearrange("p (h d) -> p h d", h=HPC),
                    bv_s[:].rearrange("p (h d) -> p h d", h=HPC),
                    ADD)

            def c_chunk(sc):
                y_s = pp.tile([P, DM], bf16, tag="ys", bufs=2,
                              name=f"ys{sc}")
                for eh in range(2):
                    psum = psS.tile([P, 2, 512], f32, tag="strip",
                                    name=f"py{sc}_{eh}")
                    for dcc in range(NEC):
                        nc.tensor.matmul(
                            psum[:, 0, :],
                            outT[dcc][:, sc * P:(sc + 1) * P],
                            wo_s[:, dcc, eh * 512:(eh + 1) * 512],
                            start=(dcc == 0), stop=(dcc == NEC - 1))
                    nc.vector.tensor_copy(
                        y_s[:, eh * 512:(eh + 1) * 512], psum[:, 0, :])
                nc.sync.dma_start(y[sc * P:(sc + 1) * P, :], y_s[:])

            fillers = []

            def pump(n=1):
                for _ in range(n):
                    if fillers:
                        fillers.pop(0)()

            # ---- prologue: QK projections for pair 0, V strips 0..3 ----
            for sb in range(NSB):
                qk_strip(0, qT, wq_s, bq_s, sb, "q")
                qk_strip(0, kT, wk_s, bk_s, sb, "k")
            for g in range(4):
                v_strip(g)

            # ---- global attention round list (software-pipelined) ----
            rounds = []
            for t in range(NEC):
                for qb in range(NQB):
                    for kc in range(4 * qb + 4):
                        rounds.append((t, qb, kc))

            def fill_for_pair(t):
                if t == 0:
                    for g in range(4, NSC):
                        fillers.append(lambda g=g: v_strip(g))
                    for sb in range(NSB):
                        fillers.append(
                            lambda sb=sb: qk_strip(1, qT, wq_s, bq_s, sb, "q"))
                        fillers.append(
                            lambda sb=sb: qk_strip(1, kT, wk_s, bk_s, sb, "k"))
                elif t < NEC - 1:
                    for sb in range(NSB):
                        fillers.append(
                            lambda sb=sb: qk_strip(t + 1, qT, wq_s, bq_s,
                                                   sb, "q"))
                        fillers.append(
                            lambda sb=sb: qk_strip(t + 1, kT, wk_s, bk_s,
                                                   sb, "k"))

            fill_for_pair(0)

            def scores_round(t, qb, kc, strip, e3):
                q0 = qb * 512
                k0 = kc * P
                off = max(0, k0 - q0)
                nc.tensor.matmul(
                    strip[:, 0, off:512],
                    kT[t][0:DK, k0:k0 + P],
                    qT[t][0:DK, q0 + off:q0 + 512],
                    start=True, stop=True)
                nc.tensor.matmul(
                    strip[:, 1, off:512],
                    kT[t][DK:P, k0:k0 + P],
                    qT[t][DK:P, q0 + off:q0 + 512],
                    start=True, stop=True)
                nc.scalar.activation(
                    e3[:, :, off:512], strip[:, :, off:512], Exp,
                    bias=pb_s[:, kc:kc + 1], scale=SCALE)
                if k0 >= q0:
                    for j in range(2):
                        nc.vector.tensor_tensor(
                            e3[:, j, off:off + P],
                            e3[:, j, off:off + P], cm_s[:], MUL)

            def attnv_round(t, qb, kc, ops, e3):
                q0 = qb * 512
                off = max(0, kc * P - q0)
                nkc = 4 * qb + 4
                for j in range(2):
                    nc.tensor.matmul(
                        ops[:, j, off:512],
                        vA[kc][:, 2 * t + j, :],
                        e3[:, j, off:512],
                        start=(kc == 0), stop=(kc == nkc - 1))

            def norm_a(t, qb, ops):
                # copy raw out^T and the denominator row off PSUM, freeing ops
                oraw = pp.tile([DK, 2, 512], bf16, tag="oraw", bufs=3,
                               name=f"or{t}_{qb}")
                nc.vector.tensor_copy(oraw[:], ops[0:DK, :, :])
                den_s = pp.tile([1, 1024], f32, tag="den", bufs=3,
                                name=f"dn{t}_{qb}")
                nc.vector.tensor_copy(den_s[:], ops[DK:DK + 1, :, :])
                return oraw, den_s

            def norm_b(t, qb, den_s):
                # reshape to [128, 8] (cheap DVE reciprocal shape) and back
                den_t = pp.tile([P, 8], f32, tag="dent", bufs=3,
                                name=f"dt{t}_{qb}")
                nc.sync.dma_start(den_t[:], den_s[0:1, :])
                rcp_t = pp.tile([P, 8], f32, tag="rcpt", bufs=3,
                                name=f"rt{t}_{qb}")
                nc.vector.reciprocal(rcp_t[:], den_t[:])
                rrow = pp.tile([1, 1024], f32, tag="rrow", bufs=3,
                               name=f"rr{t}_{qb}")
                nc.sync.dma_start(rrow[0:1, :], rcp_t[:])
                return rrow

            def norm_c(t, qb, oraw, rrow):
                # K=1 M=128 ones-matmul broadcasts 1/den across partitions
                q0 = qb * 512
                bc_ps = psO.tile([P, 2, 512], f32, tag="ops",
                                 name=f"bc{t}_{qb}")
                for j in range(2):
                    nc.tensor.matmul(
                        bc_ps[:, j, :], one1_s[:],
                        rrow[0:1, j * 512:(j + 1) * 512],
                        start=True, stop=True)
                nc.vector.tensor_tensor(
                    outT[t][0:DK, q0:q0 + 512],
                    oraw[:, 0, :], bc_ps[0:DK, 0, :], MUL)
                nc.vector.tensor_tensor(
                    outT[t][DK:P, q0:q0 + 512],
                    oraw[:, 1, :], bc_ps[DK:P, 1, :], MUL)

            # ---- main software-pipelined loop ----
            prev = None          # (t, qb, kc, ops, e3)
            ops_cur = None
            c_avail = 0          # phase-C chunks whose outT rows are done
            c_box = [0]
            pending = []         # (due_round, closure) norm stages

            def queue_c(upto):
                while c_box[0] < upto:
                    fillers.append(lambda sc=c_box[0]: c_chunk(sc))
                    c_box[0] += 1

            for ridx, (t, qb, kc) in enumerate(rounds):
                if kc == 0:
                    ops_cur = psO.tile([DK + 1, 2, 512], f32, tag="ops",
                                       name=f"op{t}_{qb}")
                strip = psS.tile([P, 2, 512], f32, tag="strip",
                                 name=f"st{t}_{qb}_{kc}")
                e3 = pp.tile([P, 2, 512], bf16, tag="exp", bufs=4,
                             name=f"ex{t}_{qb}_{kc}")
                scores_round(t, qb, kc, strip, e3)
                while pending and pending[0][0] <= ridx:
                    pending.pop(0)[1]()
                pump(1)
                if prev is not None:
                    pt, pqb, pkc, pops, pe3 = prev
                    attnv_round(pt, pqb, pkc, pops, pe3)
                    if pkc == 4 * pqb + 3:        # end of that qb
                        oraw, den_s = norm_a(pt, pqb, pops)
                        box = {}

                        def st_b(pt=pt, pqb=pqb, den_s=den_s, box=box):
                            box["rrow"] = norm_b(pt, pqb, den_s)

                        def st_c(pt=pt, pqb=pqb, oraw=oraw, box=box):
                            norm_c(pt, pqb, oraw, box["rrow"])
                            if pt == NEC - 1:
                                queue_c(4 * pqb)
                        pending.append((ridx + 2, st_b))
                        pending.append((ridx + 5, st_c))
                        if pqb == NQB - 1:        # end of that pair
                            fill_for_pair(pt + 1)
                prev = (t, qb, kc, ops_cur, e3)
            # drain the pipeline
            t, qb, kc, pops, pe3 = prev
            attnv_round(t, qb, kc, pops, pe3)
            oraw, den_s = norm_a(t, qb, pops)
            for _, f in pending:
                f()
            rrow = norm_b(t, qb, den_s)
            norm_c(t, qb, oraw, rrow)
            pump(len(fillers))
            queue_c(NSC)
            pump(len(fillers))

    _split_sync_waits(nc, mybir)
    return nc


def _prep_inputs(x, pad_mask, Wq, bq, Wk, bk, Wv, bv, Wo, bo):
    """Build the 8 per-core input maps."""
    import ml_dtypes
    bf16 = ml_dtypes.bfloat16

    def tile3(a, n):  # [n*128, F] -> [128, n, F] in bf16
        return np.ascontiguousarray(
            a.reshape(n, P, a.shape[1]).transpose(1, 0, 2).astype(bf16))

    cmv = (np.arange(P)[:, None] <= np.arange(P)[None, :]).astype(np.float32)
    in_maps = []
    for c in range(NCORES):
        b, tp = c // 2, c % 2
        sl = slice(tp * E, (tp + 1) * E)
        xT = np.ascontiguousarray(x[b].T.astype(np.float32))
        padb = np.where(pad_mask[b, 0, 0] == 1, -1e30, 0.0).astype(np.float32)
        in_maps.append({
            "xt": np.ascontiguousarray(
                tile3(xT, NDC).reshape(P, NDC, NSB, 512)
                .transpose(2, 0, 1, 3)),
            "wq": tile3(np.ascontiguousarray(Wq.T[:, sl]), NDC),
            "wk": tile3(np.ascontiguousarray(Wk.T[:, sl]), NDC),
            "wv": tile3(np.ascontiguousarray(Wv.T[:, sl]), NDC),
            "wo": tile3(np.ascontiguousarray(Wo.T[sl, :]), NEC),
            "bqt": np.ascontiguousarray(bq[sl].reshape(NEC, P).T),
            "bkt": np.ascontiguousarray(bk[sl].reshape(NEC, P).T),
            "bvb": np.ascontiguousarray(np.tile(bv[sl][None, :], (P, 1))),
            "cm": cmv,
            "one1": np.ones((1, P), dtype=np.float32),
            "pb": np.ascontiguousarray(padb.reshape(NSC, P).T),
        })
    return in_maps


def _enable_tracing():
    """Register the NTFF profile hook (the image lacks antenv.axon_hooks)
    and neuter the bucket upload the trace path attempts."""
    import sys
    import types
    try:
        import antenv.axon_hooks  # noqa: F401
    except ImportError:
        from trn_agent_boot.trn_boot import _ntff_profile_via_ctypes
        m = types.ModuleType("antenv.axon_hooks")
        hook = _ntff_profile_via_ctypes("/opt/axon/libaxon_pjrt.so")
        m.get_axon_ntff_profile_hook = lambda: hook
        sys.modules["antenv.axon_hooks"] = m
    import concourse.bass_utils as bu
    bu.upload_artifacts = lambda tmpdir: tmpdir


def kernel_with_stats(inputs, trace=False):
    from concourse.bass_utils import run_bass_kernel_spmd

    if trace:
        try:
            _enable_tracing()
        except Exception:
            trace = False

    if "nc" not in _CACHED:
        _CACHED["nc"] = _build()
    nc = _CACHED["nc"]
    in_maps = _prep_inputs(**inputs)
    res = run_bass_kernel_spmd(nc, in_maps, core_ids=list(range(NCORES)),
                               trace=trace)
    bo = inputs["bo"].astype(np.float32)
    out = np.empty((BSZ, SEQ, DM), dtype=np.float32)
    for b in range(BSZ):
        out[b] = (res.results[2 * b]["y"].astype(np.float32)
                  + res.results[2 * b + 1]["y"].astype(np.float32) + bo)
    return out, res


def kernel(**inputs):
    out, _ = kernel_with_stats(
        inputs, trace=bool(int(os.environ.get("KERNEL_TRACE", "0"))))
    return out


# revision 17
# speedup vs baseline: 1.1203x; 1.1203x over previous
# Trainium2 Bass kernel for nn_DecoderMHA (dense decoder multi-head attention).
#
# Sharding (8 NeuronCores): batch (4) x tensor-parallel over heads (2).
# Core c handles batch b = c//2 and heads [tp*8, tp*8+8) where tp = c%2,
# i.e. a 512-wide slice of the QKV projection output dim and the matching
# 512 rows of Wo^T.  Per-core partial outputs (bf16) are summed on the host
# (y[b] = part[b,0] + part[b,1] + bo).
#
# Per-core pipeline (matmul operands bf16, fp32 PSUM accumulation):
#   A) Q^T/K^T [512,2048] and V [2048,512] projections from x^T.
#   B) Attention in transposed-scores layout, processed per head PAIR
#      (the two heads sharing a [128,seq] SBUF tile at partition rows
#      0:64 / 64:128).  Score matmuls for the pair are packed into
#      concurrent PE row-groups (tile_position auto-derived from
#      base_partition 0/64) and land in one [128,2,512] PSUM strip;
#      ScalarE applies exp(s/8 + pad_bias) for both heads in a single
#      activation op.  attn@V uses a ones-augmented V (65th column) so
#      PSUM row 64 of the [65,2,512] out tile is the softmax denominator.
#      Normalisation is fully on-chip: DVE copies the denominator rows to
#      SBUF, one stride-0 broadcast DMA replicates them across partitions,
#      and a DVE divide writes the normalised out^T in bf16.
#      Rounds are software-pipelined (scores of round r+1 issue before
#      attn@V of round r) so ScalarE - the attention-phase bottleneck -
#      never starves, and projection / output-projection work is pumped
#      into the PE gaps as fillers.
#   C) y = out @ Wo^T partial via out^T-as-lhsT matmuls, bf16 output.
import os
import numpy as np

BSZ, SEQ, DM = 4, 2048, 1024
HEADS, DK = 16, 64
NCORES, TP = 8, 2
E = DM // TP          # 512 per-core projection slice
HPC = HEADS // TP     # 8 heads per core
P = 128
NDC = DM // P         # 8 contraction chunks
NEC = E // P          # 4 e-chunks (head pairs)
NSC = SEQ // P        # 16 sequence chunks
NSB = SEQ // 512      # 4 sequence blocks
NQB = SEQ // 512      # 4 query blocks of 512
SCALE = 1.0 / float(np.sqrt(DK))

_CACHED = {}


def _split_sync_waits(nc, mybir, max_waits=1):
    """The walrus in this container only accepts one sync-wait per
    instruction; move excess waits onto NoOps in front."""
    n = 0
    for fn in nc.m.functions:
        for bb in fn.blocks:
            insts = bb.instructions
            i = 0
            while i < len(insts):
                inst = insts[i]
                si = getattr(inst, "sync_info", None)
                if si is not None and si.on_wait and len(si.on_wait) > max_waits:
                    waits = list(si.on_wait)
                    extra, keep = waits[:-max_waits], waits[-max_waits:]
                    si.on_wait = keep
                    pos = i
                    for j in range(0, len(extra), max_waits):
                        nop = mybir.InstNoOp(
                            name=nc.get_next_instruction_name(),
                            sync_info=mybir.SyncInfo(
                                on_wait=extra[j:j + max_waits], on_update=[]),
                            bass_nofuse=True,
                            engine=inst.engine,
                        )
                        insts.insert(pos, nop)
                        pos += 1
                        i += 1
                        n += 1
                i += 1
    return n


def _build():
    import concourse.bass as bass
    from concourse import mybir
    from concourse.tile import TileContext

    f32 = mybir.dt.float32
    bf16 = mybir.dt.bfloat16
    Exp = mybir.ActivationFunctionType.Exp
    MUL = mybir.AluOpType.mult
    ADD = mybir.AluOpType.add
    DIV = mybir.AluOpType.divide

    nc = bass.Bass("TRN2", target_bir_lowering=False, debug=False,
                   num_devices=NCORES)

    # DRAM I/O (per-core layouts, pre-tiled on host)
    xt = nc.dram_tensor("xt", [NSB, P, NDC, 512], bf16, kind="ExternalInput")
    wq = nc.dram_tensor("wq", [P, NDC, E], bf16, kind="ExternalInput")
    wk = nc.dram_tensor("wk", [P, NDC, E], bf16, kind="ExternalInput")
    wv = nc.dram_tensor("wv", [P, NDC, E], bf16, kind="ExternalInput")
    wo = nc.dram_tensor("wo", [P, NEC, DM], bf16, kind="ExternalInput")
    bqt = nc.dram_tensor("bqt", [P, NEC], f32, kind="ExternalInput")
    bkt = nc.dram_tensor("bkt", [P, NEC], f32, kind="ExternalInput")
    bvb = nc.dram_tensor("bvb", [P, E], f32, kind="ExternalInput")
    cm = nc.dram_tensor("cm", [P, P], f32, kind="ExternalInput")
    pb = nc.dram_tensor("pb", [P, NSC], f32, kind="ExternalInput")
    one1 = nc.dram_tensor("one1", [1, P], bf16, kind="ExternalInput")
    y = nc.dram_tensor("y", [SEQ, DM], bf16, kind="ExternalOutput")

    with TileContext(nc) as tc:
        with (
            tc.tile_pool(name="persist", bufs=1) as pp,
            tc.tile_pool(name="psS", bufs=2, space="PSUM") as psS,
            tc.tile_pool(name="psO", bufs=2, space="PSUM") as psO,
        ):
            # ---- persistent SBUF ----
            qT = [pp.tile([P, SEQ], bf16, tag=f"qT{t}", name=f"qT{t}")
                  for t in range(NEC)]
            kT = [pp.tile([P, SEQ], bf16, tag=f"kT{t}", name=f"kT{t}")
                  for t in range(NEC)]
            vA = [pp.tile([P, HPC, DK + 1], bf16, tag=f"vA{g}",
                          name=f"vA{g}") for g in range(NSC)]
            outT = [pp.tile([P, SEQ], bf16, tag=f"oT{t}", name=f"oT{t}")
                    for t in range(NEC)]
            cm_s = pp.tile([P, P], f32, tag="cm")
            one1_s = pp.tile([1, P], bf16, tag="one1")
            pb_s = pp.tile([P, NSC], f32, tag="pb")
            bq_s = pp.tile([P, NEC], f32, tag="bq")
            bk_s = pp.tile([P, NEC], f32, tag="bk")
            bv_s = pp.tile([P, E], f32, tag="bv")
            wq_s = pp.tile([P, NDC, E], bf16, tag="wq")
            wk_s = pp.tile([P, NDC, E], bf16, tag="wk")
            wv_s = pp.tile([P, NDC, E], bf16, tag="wv")
            wo_s = pp.tile([P, NEC, DM], bf16, tag="wo")
            xt_s = [pp.tile([P, NDC, 512], bf16, tag=f"xt{sb}",
                            name=f"xt{sb}") for sb in range(NSB)]

            # ---- input DMAs, in consumption order ----
            nc.sync.dma_start(wq_s[:], wq[:])
            nc.sync.dma_start(xt_s[0][:], xt[0])
            nc.sync.dma_start(wk_s[:], wk[:])
            nc.sync.dma_start(bq_s[:], bqt[:])
            nc.sync.dma_start(bk_s[:], bkt[:])
            nc.sync.dma_start(pb_s[:], pb[:])
            nc.sync.dma_start(cm_s[:], cm[:])
            nc.sync.dma_start(one1_s[:], one1[:])
            for sb in range(1, NSB):
                nc.sync.dma_start(xt_s[sb][:], xt[sb])
            nc.sync.dma_start(wv_s[:], wv[:])
            nc.sync.dma_start(bv_s[:], bvb[:])
            for g in range(NSC):
                nc.gpsimd.memset(vA[g][:, :, DK:DK + 1], 1.0)
            nc.sync.dma_start(wo_s[:], wo[:])

            # ---- emit helpers ----
            def qk_strip(t, dst, w_s, b_s, sb, nm):
                psum = psS.tile([P, 2, 512], f32, tag="strip",
                                name=f"p{nm}{t}_{sb}")
                for dc in range(NDC):
                    nc.tensor.matmul(
                        psum[:, 0, :],
                        w_s[:, dc, t * P:(t + 1) * P],
                        xt_s[sb][:, dc, :],
                        start=(dc == 0), stop=(dc == NDC - 1))
                nc.vector.tensor_tensor(
                    dst[t][:, sb * 512:(sb + 1) * 512],
                    psum[:, 0, :],
                    b_s[:, t:t + 1].to_broadcast([P, 512]),
                    ADD)

            def v_strip(g):
                sb, ssc = g // 4, g % 4
                psum = psS.tile([P, 2, 512], f32, tag="strip", name=f"pv{g}")
                for dc in range(NDC):
                    nc.tensor.matmul(
                        psum[:, 0, :],
                        xt_s[sb][:, dc, ssc * P:(ssc + 1) * P],
                        wv_s[:, dc, :],
                        start=(dc == 0), stop=(dc == NDC - 1))
                nc.vector.tensor_tensor(
                    vA[g][:, :, 0:DK],
                    psum[:, 0, :].rearrange("p (h d) -> p h d", h=HPC),
                    bv_s[:].rearrange("p (h d) -> p h d", h=HPC),
                    ADD)

            def c_chunk(sc):
                y_s = pp.tile([P, DM], bf16, tag="ys", bufs=2,
                              name=f"ys{sc}")
                for eh in range(2):
                    psum = psS.tile([P, 2, 512], f32, tag="strip",
                                    name=f"py{sc}_{eh}")
                    for dcc in range(NEC):
                        nc.tensor.matmul(
                            psum[:, 0, :],
                            outT[dcc][:, sc * P:(sc + 1) * P],
                            wo_s[:, dcc, eh * 512:(eh + 1) * 512],
                            start=(dcc == 0), stop=(dcc == NEC - 1))
                    nc.vector.tensor_copy(
                        y_s[:, eh * 512:(eh + 1) * 512], psum[:, 0, :])
                nc.sync.dma_start(y[sc * P:(sc + 1) * P, :], y_s[:])

            fillers = []

            def pump(n=1):
                for _ in range(n):
                    if fillers:
                        fillers.pop(0)()

            # ---- prologue: QK projections for pair 0, V strips 0..3 ----
            for sb in range(NSB):
                qk_strip(0, qT, wq_s, bq_s, sb, "q")
                qk_strip(0, kT, wk_s, bk_s, sb, "k")
            for g in range(4):
                v_strip(g)

            # ---- global attention round list (software-pipelined) ----
            rounds = []
            for t in range(NEC):
                for qb in range(NQB):
                    for kc in range(4 * qb + 4):
                        rounds.append((t, qb, kc))

            def fill_for_pair(t):
                if t == 0:
                    for g in range(4, NSC):
                        fillers.append(lambda g=g: v_strip(g))
                    for sb in range(NSB):
                        fillers.append(
                            lambda sb=sb: qk_strip(1, qT, wq_s, bq_s, sb, "q"))
                        fillers.append(
                            lambda sb=sb: qk_strip(1, kT, wk_s, bk_s, sb, "k"))
                elif t < NEC - 1:
                    for sb in range(NSB):
                        fillers.append(
                            lambda sb=sb: qk_strip(t + 1, qT, wq_s, bq_s,
                                                   sb, "q"))
                        fillers.append(
                            lambda sb=sb: qk_strip(t + 1, kT, wk_s, bk_s,
                                                   sb, "k"))

            fill_for_pair(0)

            def scores_round(t, qb, kc, strip, e3):
                q0 = qb * 512
                k0 = kc * P
                off = max(0, k0 - q0)
                nc.tensor.matmul(
                    strip[:, 0, off:512],
                    kT[t][0:DK, k0:k0 + P],
                    qT[t][0:DK, q0 + off:q0 + 512],
                    start=True, stop=True)
                nc.tensor.matmul(
                    strip[:, 1, off:512],
                    kT[t][DK:P, k0:k0 + P],
                    qT[t][DK:P, q0 + off:q0 + 512],
                    start=True, stop=True)
                nc.scalar.activation(
                    e3[:, :, off:512], strip[:, :, off:512], Exp,
                    bias=pb_s[:, kc:kc + 1], scale=SCALE)
                if k0 >= q0:
                    for j in range(2):
                        nc.vector.tensor_tensor(
                            e3[:, j, off:off + P],
                            e3[:, j, off:off + P], cm_s[:], MUL)

            def attnv_round(t, qb, kc, ops, e3):
                q0 = qb * 512
                off = max(0, kc * P - q0)
                nkc = 4 * qb + 4
                for j in range(2):
                    nc.tensor.matmul(
                        ops[:, j, off:512],
                        vA[kc][:, 2 * t + j, :],
                        e3[:, j, off:512],
                        start=(kc == 0), stop=(kc == nkc - 1))

            def norm_a(t, qb, ops):
                # copy raw out^T and the denominator row off PSUM, freeing ops
                oraw = pp.tile([DK, 2, 512], bf16, tag="oraw", bufs=3,
                               name=f"or{t}_{qb}")
                nc.vector.tensor_copy(oraw[:], ops[0:DK, :, :])
                den_s = pp.tile([1, 1024], f32, tag="den", bufs=3,
                                name=f"dn{t}_{qb}")
                nc.vector.tensor_copy(den_s[:], ops[DK:DK + 1, :, :])
                return oraw, den_s

            def norm_b(t, qb, den_s):
                # reshape to [128, 8] (cheap DVE reciprocal shape) and back
                den_t = pp.tile([P, 8], f32, tag="dent", bufs=3,
                                name=f"dt{t}_{qb}")
                nc.sync.dma_start(den_t[:], den_s[0:1, :])
                rcp_t = pp.tile([P, 8], f32, tag="rcpt", bufs=3,
                                name=f"rt{t}_{qb}")
                nc.vector.reciprocal(rcp_t[:], den_t[:])
                rrow = pp.tile([1, 1024], bf16, tag="rrow", bufs=3,
                               name=f"rr{t}_{qb}")
                nc.gpsimd.dma_start(rrow[0:1, :], rcp_t[:])
                return rrow

            def norm_c(t, qb, oraw, rrow):
                # K=1 M=128 ones-matmul broadcasts 1/den across partitions
                q0 = qb * 512
                bc_ps = psO.tile([P, 2, 512], f32, tag="ops",
                                 name=f"bc{t}_{qb}")
                for j in range(2):
                    nc.tensor.matmul(
                        bc_ps[:, j, :], one1_s[:],
                        rrow[0:1, j * 512:(j + 1) * 512],
                        start=True, stop=True)
                nc.vector.tensor_tensor(
                    outT[t][0:DK, q0:q0 + 512],
                    oraw[:, 0, :], bc_ps[0:DK, 0, :], MUL)
                nc.vector.tensor_tensor(
                    outT[t][DK:P, q0:q0 + 512],
                    oraw[:, 1, :], bc_ps[DK:P, 1, :], MUL)

            # ---- main software-pipelined loop ----
            prev = None          # (t, qb, kc, ops, e3)
            ops_cur = None
            c_avail = 0          # phase-C chunks whose outT rows are done
            c_box = [0]
            pending = []         # (due_round, closure) norm stages

            def queue_c(upto):
                while c_box[0] < upto:
                    fillers.append(lambda sc=c_box[0]: c_chunk(sc))
                    c_box[0] += 1

            for ridx, (t, qb, kc) in enumerate(rounds):
                if kc == 0:
                    ops_cur = psO.tile([DK + 1, 2, 512], f32, tag="ops",
                                       name=f"op{t}_{qb}")
                strip = psS.tile([P, 2, 512], f32, tag="strip",
                                 name=f"st{t}_{qb}_{kc}")
                e3 = pp.tile([P, 2, 512], bf16, tag="exp", bufs=4,
                             name=f"ex{t}_{qb}_{kc}")
                scores_round(t, qb, kc, strip, e3)
                while pending and pending[0][0] <= ridx:
                    pending.pop(0)[1]()
                pump(1)
                if prev is not None:
                    pt, pqb, pkc, pops, pe3 = prev
                    attnv_round(pt, pqb, pkc, pops, pe3)
                    if pkc == 4 * pqb + 3:        # end of that qb
                        oraw, den_s = norm_a(pt, pqb, pops)
                        box = {}

                        def st_b(pt=pt, pqb=pqb, den_s=den_s, box=box):
                            box["rrow"] = norm_b(pt, pqb, den_s)

                        def st_c(pt=pt, pqb=pqb, oraw=oraw, box=box):
                            norm_c(pt, pqb, oraw, box["rrow"])
                            if pt == NEC - 1:
                                queue_c(4 * pqb)
                        pending.append((ridx + 2, st_b))
                        pending.append((ridx + 5, st_c))
                        if pqb == NQB - 1:        # end of that pair
                            fill_for_pair(pt + 1)
                prev = (t, qb, kc, ops_cur, e3)
            # drain the pipeline
            t, qb, kc, pops, pe3 = prev
            attnv_round(t, qb, kc, pops, pe3)
            oraw, den_s = norm_a(t, qb, pops)
            for _, f in pending:
                f()
            rrow = norm_b(t, qb, den_s)
            norm_c(t, qb, oraw, rrow)
            pump(len(fillers))
            queue_c(NSC)
            pump(len(fillers))

    _split_sync_waits(nc, mybir)
    return nc


def _prep_inputs(x, pad_mask, Wq, bq, Wk, bk, Wv, bv, Wo, bo):
    """Build the 8 per-core input maps."""
    import ml_dtypes
    bf16 = ml_dtypes.bfloat16

    def tile3(a, n):  # [n*128, F] -> [128, n, F] in bf16
        return np.ascontiguousarray(
            a.reshape(n, P, a.shape[1]).transpose(1, 0, 2).astype(bf16))

    cmv = (np.arange(P)[:, None] <= np.arange(P)[None, :]).astype(np.float32)
    in_maps = []
    for c in range(NCORES):
        b, tp = c // 2, c % 2
        sl = slice(tp * E, (tp + 1) * E)
        xT = np.ascontiguousarray(x[b].T.astype(np.float32))
        padb = np.where(pad_mask[b, 0, 0] == 1, -1e30, 0.0).astype(np.float32)
        in_maps.append({
            "xt": np.ascontiguousarray(
                tile3(xT, NDC).reshape(P, NDC, NSB, 512)
                .transpose(2, 0, 1, 3)),
            "wq": tile3(np.ascontiguousarray(Wq.T[:, sl]), NDC),
            "wk": tile3(np.ascontiguousarray(Wk.T[:, sl]), NDC),
            "wv": tile3(np.ascontiguousarray(Wv.T[:, sl]), NDC),
            "wo": tile3(np.ascontiguousarray(Wo.T[sl, :]), NEC),
            "bqt": np.ascontiguousarray(bq[sl].reshape(NEC, P).T),
            "bkt": np.ascontiguousarray(bk[sl].reshape(NEC, P).T),
            "bvb": np.ascontiguousarray(np.tile(bv[sl][None, :], (P, 1))),
            "cm": cmv,
            "one1": np.ones((1, P), dtype=bf16),
            "pb": np.ascontiguousarray(padb.reshape(NSC, P).T),
        })
    return in_maps


def _enable_tracing():
    """Register the NTFF profile hook (the image lacks antenv.axon_hooks)
    and neuter the bucket upload the trace path attempts."""
    import sys
    import types
    try:
        import antenv.axon_hooks  # noqa: F401
    except ImportError:
        from trn_agent_boot.trn_boot import _ntff_profile_via_ctypes
        m = types.ModuleType("antenv.axon_hooks")
        hook = _ntff_profile_via_ctypes("/opt/axon/libaxon_pjrt.so")
        m.get_axon_ntff_profile_hook = lambda: hook
        sys.modules["antenv.axon_hooks"] = m
    import concourse.bass_utils as bu
    bu.upload_artifacts = lambda tmpdir: tmpdir


def kernel_with_stats(inputs, trace=False):
    from concourse.bass_utils import run_bass_kernel_spmd

    if trace:
        try:
            _enable_tracing()
        except Exception:
            trace = False

    if "nc" not in _CACHED:
        _CACHED["nc"] = _build()
    nc = _CACHED["nc"]
    in_maps = _prep_inputs(**inputs)
    res = run_bass_kernel_spmd(nc, in_maps, core_ids=list(range(NCORES)),
                               trace=trace)
    bo = inputs["bo"].astype(np.float32)
    out = np.empty((BSZ, SEQ, DM), dtype=np.float32)
    for b in range(BSZ):
        out[b] = (res.results[2 * b]["y"].astype(np.float32)
                  + res.results[2 * b + 1]["y"].astype(np.float32) + bo)
    return out, res


def kernel(**inputs):
    out, _ = kernel_with_stats(
        inputs, trace=bool(int(os.environ.get("KERNEL_TRACE", "0"))))
    return out
